# revision 8
# baseline (speedup 1.0000x reference)
"""Trainium2 Bass kernel for nn_BaseSparseVAE (B=512, D=512, L=32, H=300).

Strategy (8 NeuronCores):
  - Shard the decoder's per-column axis D=512 across cores: 64 columns each,
    processed as 16 groups of 4 columns.
  - Encoder is tiny; replicate it on every core (no collectives needed).
  - Everything runs in transposed [feature, batch] layout so biases/masks are
    per-partition and matmuls are lhsT.T @ rhs with batch as the moving dim.
  - Matmul dtypes: encoder in float32r (accuracy for the z outputs), decoder
    in fp16 (1 cyc/row streaming + tile_position packing of the 4 columns).
  - Relu/bias work is split between the Scalar (ACT) and Vector (DVE) engines.

Self-contained: includes the walrus single-sync-wait workarounds inline.
"""

import numpy as np

import concourse.bass as bass
import concourse.mybir as mybir
from concourse.tile import TileContext
from concourse.vector_clock import ScopedClock
from concourse.bass_utils import run_bass_kernel_spmd

FP32 = mybir.dt.float32
F32R = mybir.dt.float32r
FP16 = mybir.dt.float16
AF = mybir.ActivationFunctionType
ALU = mybir.AluOpType

B, D, L, H = 512, 512, 32, 300
NCORES = 8
DCORE = D // NCORES          # 64 columns per core
NGROUPS = DCORE // 4         # 16 groups of 4 columns
MC = [0, 128, 256, 300]      # H chunk boundaries (partition chunks)
DC = [0, 128, 256, 384, 512]  # D chunk boundaries

# ---------------------------------------------------------------------------
# walrus workarounds: this toolchain accepts only ONE inline sync wait per
# instruction. Split excess waits onto same-engine EventSemaphore carriers.
# ---------------------------------------------------------------------------

def _patched_drain_and_barrier(self, tick_clock, wait_clock):
    nc = self.nc
    drain_inst = nc.sync.drain()
    wait_clock.add_sem_waits(
        drain_inst.ins, ScopedClock({None: tick_clock.global_clock})
    )
    si = drain_inst.ins.sync_info
    waits = list(si.on_wait) if si and si.on_wait else []
    if len(waits) > 1:
        del si.on_wait[1:]
        for w in waits[1:]:
            extra = nc.sync.drain()
            extra.ins.sync_info = mybir.SyncInfo(on_wait=[w], on_update=[])
    nc.all_engine_barrier()
    popped = nc._tile_sem_poison_stack.pop()
    assert popped is self._sem_poison
    nc.clear_and_free_semaphores(list(self.sems.allocated().values()))
    nc.all_engine_barrier()


TileContext._drain_and_barrier = _patched_drain_and_barrier

_carrier_ctr = [0]


def _split_sync_waits(nc, max_inline=1):
    moved = 0
    for f in nc.m.functions:
        for blk in f.blocks:
            new = []
            changed = False
            for inst in blk.instructions:
                si = inst.sync_info
                waits = list(si.on_wait) if si and si.on_wait else []
                if len(waits) > max_inline:
                    for w in waits[max_inline:]:
                        _carrier_ctr[0] += 1
                        new.append(mybir.InstEventSemaphore(
                            name=f"waitsplit_{_carrier_ctr[0]}",
                            ins=[], outs=[], engine=inst.engine,
                            sync_info=mybir.SyncInfo(on_wait=[w], on_update=[]),
                        ))
                        moved += 1
                    del si.on_wait[max_inline:]
                    changed = True
                new.append(inst)
            if changed:
                blk.instructions = new
    return moved


# ---------------------------------------------------------------------------
# program builder
# ---------------------------------------------------------------------------

def _build_program():
    nc = bass.Bass("TRN2")

    # inputs (per-core views; most are identical on all cores)
    xt = nc.dram_tensor("xt", [D, B], F32R, kind="ExternalInput")
    epst = nc.dram_tensor("epst", [L, B], FP32, kind="ExternalInput")
    qzw1 = nc.dram_tensor("qzw1", [D, H], F32R, kind="ExternalInput")
    qzb1 = nc.dram_tensor("qzb1", [H, 1], FP32, kind="ExternalInput")
    qzw2 = nc.dram_tensor("qzw2", [H, H], F32R, kind="ExternalInput")
    qzb2 = nc.dram_tensor("qzb2", [H, 1], FP32, kind="ExternalInput")
    zmw = nc.dram_tensor("zmw", [H, L], F32R, kind="ExternalInput")
    zmb = nc.dram_tensor("zmb", [L, 1], FP32, kind="ExternalInput")
    zlw = nc.dram_tensor("zlw", [H, L], F32R, kind="ExternalInput")
    zlb = nc.dram_tensor("zlb", [L, 1], FP32, kind="ExternalInput")
    hzlb = nc.dram_tensor("hzlb", [L, 1], FP32, kind="ExternalInput")  # 0.5*zl_b
    w1r = nc.dram_tensor("w1r", [128, H], FP16, kind="ExternalInput")  # gen_w1 4-rep
    w2 = nc.dram_tensor("w2", [H, H], FP16, kind="ExternalInput")
    b2 = nc.dram_tensor("b2", [H, 1], FP32, kind="ExternalInput")
    w4 = nc.dram_tensor("w4", [128, NGROUPS], FP32, kind="ExternalInput")   # per-core
    # col_w in fat zero-padded blocks: per column j a [H, 32] block whose only
    # nonzero col is r = (j//4) % 4 -> final matmuls write M=32 rows per strip,
    # accumulating 4 groups into one PSUM bank before a single copy-out.
    cwf = nc.dram_tensor("cwf", [H, DCORE * 32], FP16, kind="ExternalInput")  # per-core
    cbf = nc.dram_tensor("cbf", [128, NGROUPS // 4], FP32, kind="ExternalInput")  # per-core

    # outputs
    xpart = nc.dram_tensor("xpart", [DCORE, B], FP32, kind="ExternalOutput")
    zt_o = nc.dram_tensor("zt_o", [L, B], FP32, kind="ExternalOutput")
    zmt_o = nc.dram_tensor("zmt_o", [L, B], FP32, kind="ExternalOutput")
    zlvt_o = nc.dram_tensor("zlvt_o", [L, B], FP32, kind="ExternalOutput")

    with TileContext(nc) as tc:
        with (
            tc.tile_pool(name="wsb", bufs=1) as wsb,      # weights + persistent
            tc.tile_pool(name="hsb", bufs=1) as hsb,      # encoder activations
            tc.tile_pool(name="dec", bufs=2) as dec,      # decoder activations
        ):
            # ---- weight loads ----
            txt = []
            for ki in range(4):
                t = wsb.tile([128, B], F32R, tag=f"xt{ki}")
                nc.sync.dma_start(t[:, :], xt[DC[ki]:DC[ki + 1], :])
                txt.append(t)
            tq1 = {}
            for ki in range(4):
                for mi in range(3):
                    mc = MC[mi + 1] - MC[mi]
                    t = wsb.tile([128, mc], F32R, tag=f"q1_{ki}_{mi}")
                    nc.sync.dma_start(t[:, :], qzw1[DC[ki]:DC[ki + 1], MC[mi]:MC[mi + 1]])
                    tq1[ki, mi] = t
            tq2 = {}
            tw2 = {}
            for ki in range(3):
                kc = MC[ki + 1] - MC[ki]
                for mi in range(3):
                    mc = MC[mi + 1] - MC[mi]
                    t = wsb.tile([kc, mc], F32R, tag=f"q2_{ki}_{mi}")
                    nc.sync.dma_start(t[:, :], qzw2[MC[ki]:MC[ki + 1], MC[mi]:MC[mi + 1]])
                    tq2[ki, mi] = t
                    t = wsb.tile([kc, mc], FP16, tag=f"w2_{ki}_{mi}")
                    nc.sync.dma_start(t[:, :], w2[MC[ki]:MC[ki + 1], MC[mi]:MC[mi + 1]])
                    tw2[ki, mi] = t
            tzmw, tzlw, tcw = {}, {}, {}
            for ki in range(3):
                kc = MC[ki + 1] - MC[ki]
                t = wsb.tile([kc, L], F32R, tag=f"zmw{ki}")
                nc.sync.dma_start(t[:, :], zmw[MC[ki]:MC[ki + 1], :])
                tzmw[ki] = t
                t = wsb.tile([kc, L], F32R, tag=f"zlw{ki}")
                nc.sync.dma_start(t[:, :], zlw[MC[ki]:MC[ki + 1], :])
                tzlw[ki] = t
                t = wsb.tile([kc, DCORE * 32], FP16, tag=f"cw{ki}")
                nc.sync.dma_start(t[:, :], cwf[MC[ki]:MC[ki + 1], :])
                tcw[ki] = t
            tb1, tb2q, tb2 = {}, {}, {}
            for mi in range(3):
                mc = MC[mi + 1] - MC[mi]
                t = wsb.tile([mc, 1], FP32, tag=f"b1_{mi}")
                nc.sync.dma_start(t[:, :], qzb1[MC[mi]:MC[mi + 1], :])
                tb1[mi] = t
                t = wsb.tile([mc, 1], FP32, tag=f"b2q_{mi}")
                nc.sync.dma_start(t[:, :], qzb2[MC[mi]:MC[mi + 1], :])
                tb2q[mi] = t
                t = wsb.tile([mc, 1], FP32, tag=f"b2_{mi}")
                nc.sync.dma_start(t[:, :], b2[MC[mi]:MC[mi + 1], :])
                tb2[mi] = t
            tzmb = wsb.tile([L, 1], FP32, tag="zmb")
            nc.sync.dma_start(tzmb[:, :], zmb[:, :])
            tzlb = wsb.tile([L, 1], FP32, tag="zlb")
            nc.sync.dma_start(tzlb[:, :], zlb[:, :])
            thzlb = wsb.tile([L, 1], FP32, tag="hzlb")
            nc.sync.dma_start(thzlb[:, :], hzlb[:, :])
            teps = wsb.tile([L, B], FP32, tag="eps")
            nc.sync.dma_start(teps[:, :], epst[:, :])
            tw1 = wsb.tile([128, H], FP16, tag="w1r")
            nc.sync.dma_start(tw1[:, :], w1r[:, :])
            tw4 = wsb.tile([128, NGROUPS], FP32, tag="w4")
            nc.sync.dma_start(tw4[:, :], w4[:, :])
            tcb = wsb.tile([128, NGROUPS // 4], FP32, tag="cbf")
            nc.sync.dma_start(tcb[:, :], cbf[:, :])

            # ---- encoder ----
            with tc.tile_pool(name="eps_ps", bufs=1, space="PSUM") as eps_ps:
                h1 = {}
                for mi in range(3):
                    mc = MC[mi + 1] - MC[mi]
                    pt = eps_ps.tile([128, B], FP32, tag=f"ph1_{mi}")
                    for ki in range(4):
                        nc.tensor.matmul(pt[0:mc, :], tq1[ki, mi][:, :], txt[ki][:, :],
                                         start=(ki == 0), stop=(ki == 3))
                    t = hsb.tile([128, B], F32R, tag=f"h1_{mi}")
                    nc.scalar.activation(t[0:mc, :], pt[0:mc, :], AF.Relu,
                                         bias=tb1[mi][:, :])
                    h1[mi] = t
                h2 = {}
                for mi in range(3):
                    mc = MC[mi + 1] - MC[mi]
                    pt = eps_ps.tile([128, B], FP32, tag=f"ph2_{mi}")
                    for ki in range(3):
                        kc = MC[ki + 1] - MC[ki]
                        nc.tensor.matmul(pt[0:mc, :], tq2[ki, mi][:, :], h1[ki][0:kc, :],
                                         start=(ki == 0), stop=(ki == 2))
                    t = hsb.tile([128, B], F32R, tag=f"h2_{mi}")
                    nc.scalar.activation(t[0:mc, :], pt[0:mc, :], AF.Relu,
                                         bias=tb2q[mi][:, :])
                    h2[mi] = t
                # z heads
                pzm = eps_ps.tile([L, B], FP32, tag="pzm")
                pzl = eps_ps.tile([L, B], FP32, tag="pzl")
                for ki in range(3):
                    kc = MC[ki + 1] - MC[ki]
                    nc.tensor.matmul(pzm[:, :], tzmw[ki][:, :], h2[ki][0:kc, :],
                                     start=(ki == 0), stop=(ki == 2))
                for ki in range(3):
                    kc = MC[ki + 1] - MC[ki]
                    nc.tensor.matmul(pzl[:, :], tzlw[ki][:, :], h2[ki][0:kc, :],
                                     start=(ki == 0), stop=(ki == 2))
                tzm = hsb.tile([L, B], FP32, tag="tzm")
                nc.scalar.activation(tzm[:, :], pzm[:, :], AF.Identity, bias=tzmb[:, :])
                tzlv = hsb.tile([L, B], FP32, tag="tzlv")
                nc.scalar.activation(tzlv[:, :], pzl[:, :], AF.Identity, bias=tzlb[:, :])
                texp = hsb.tile([L, B], FP32, tag="texp")
                nc.scalar.activation(texp[:, :], pzl[:, :], AF.Exp, bias=thzlb[:, :],
                                     scale=0.5)
                # z = z_mean + eps * exp(0.5*zlv)
                tz1 = hsb.tile([L, B], FP32, tag="tz1")
                nc.vector.tensor_tensor(tz1[:, :], teps[:, :], texp[:, :],
                                        op=ALU.mult)
                nc.vector.tensor_tensor(tz1[:, :], tz1[:, :], tzm[:, :],
                                        op=ALU.add)
                # outputs
                nc.sync.dma_start(zt_o[:, :], tz1[:, :])
                nc.sync.dma_start(zmt_o[:, :], tzm[:, :])
                nc.sync.dma_start(zlvt_o[:, :], tzlv[:, :])
                # z4: 4 stacked copies of zT
                tz4 = hsb.tile([128, B], FP32, tag="tz4")
                for i in range(4):
                    nc.sync.dma_start(tz4[32 * i:32 * i + 32, :], tz1[:, :])

            # ---- decoder ----
            relu_tick = [0]

            def relu_split(dst, src, bias=None):
                # ACT:DVE = 13:11 per 24 relu ops (DVE also owns masks + copies)
                k = relu_tick[0] % 24
                relu_tick[0] += 1
                if k < 13:
                    if bias is None:
                        nc.scalar.activation(dst, src, AF.Relu)
                    else:
                        nc.scalar.activation(dst, src, AF.Relu, bias=bias)
                else:
                    if bias is None:
                        nc.vector.tensor_scalar_max(dst, src, 0.0)
                    else:
                        nc.vector.tensor_scalar(dst, src, bias, 0.0,
                                                op0=ALU.add, op1=ALU.max)

            with (
                tc.tile_pool(name="dps", bufs=1, space="PSUM") as dps,
                tc.tile_pool(name="fps", bufs=1, space="PSUM") as fps,
            ):
                for G in range(NGROUPS // 4):
                    finb = [fps.tile([128, B], FP32, tag=f"fin{i}", name=f"fin_{G}_{i}")
                            for i in range(4)]
                    for gs in range(4):
                        g = 4 * G + gs
                        msk = dec.tile([128, B], FP16, tag="msk")
                        nc.vector.tensor_scalar_mul(msk[:, :], tz4[:, :], tw4[:, g:g + 1])

                        # L1: 4-row-packed fp16 (K=32 each); g1 free-blocked by mi
                        g1 = [dec.tile([128, 3 * B], FP16, tag=f"g1_{i}", name=f"g1t_{g}_{i}")
                              for i in range(4)]
                        for mi in range(3):
                            mc = MC[mi + 1] - MC[mi]
                            pts = []
                            for i in range(4):
                                pt = dps.tile([128, B], FP32, tag=f"w{i}")
                                kw = {}
                                if i == 3:
                                    kw["tile_position"] = (96, 0)
                                nc.tensor.matmul(pt[0:mc, :],
                                                 tw1[32 * i:32 * i + 32, MC[mi]:MC[mi + 1]],
                                                 msk[32 * i:32 * i + 32, :],
                                                 start=True, stop=True, **kw)
                                pts.append(pt)
                            for i in range(4):
                                relu_split(g1[i][0:mc, B * mi:B * mi + B], pts[i][0:mc, :])

                        # L2: fp16, weights shared across the 4 columns (i innermost)
                        g2 = [dec.tile([128, 3 * B], FP16, tag=f"g2_{i}", name=f"g2t_{g}_{i}")
                              for i in range(4)]
                        for mi in range(3):
                            mc = MC[mi + 1] - MC[mi]
                            pts = [dps.tile([128, B], FP32, tag=f"w{i}", name=f"l2p_{g}_{mi}_{i}")
                                   for i in range(4)]
                            for ki in range(3):
                                kc = MC[ki + 1] - MC[ki]
                                for i in range(4):
                                    nc.tensor.matmul(pts[i][0:mc, :], tw2[ki, mi][:, :],
                                                     g1[i][0:kc, B * ki:B * ki + B],
                                                     start=(ki == 0), stop=(ki == 2))
                            for i in range(4):
                                relu_split(g2[i][0:mc, B * mi:B * mi + B], pts[i][0:mc, :],
                                           bias=tb2[mi][:, :])

                        # final: fp16 matvecs, strip i -> own bank, M=32 rows with
                        # zero-padded lhsT; 4 groups accumulate before copy-out
                        for ki in range(3):
                            kc = MC[ki + 1] - MC[ki]
                            for i in range(4):
                                jloc = 4 * g + i
                                kw = {}
                                if i == 3:
                                    kw["tile_position"] = (0, 96)
                                nc.tensor.matmul(finb[i][32 * i:32 * i + 32, :],
                                                 tcw[ki][:, 32 * jloc:32 * jloc + 32],
                                                 g2[i][0:kc, B * ki:B * ki + B],
                                                 start=(gs == 0 and ki == 0),
                                                 stop=(gs == 3 and ki == 2), **kw)

                    # copy out the 4 accumulated banks (+ col_b) and store
                    xr = dec.tile([128, B], FP32, tag="xr", name=f"xr_{G}")
                    for i in range(4):
                        nc.vector.tensor_scalar_add(
                            xr[32 * i:32 * i + 4, :], finb[i][32 * i:32 * i + 4, :],
                            tcb[32 * i:32 * i + 4, G:G + 1])
                        nc.sync.dma_start(xpart[16 * G + i:16 * G + 16:4, :],
                                          xr[32 * i:32 * i + 4, :])

    _split_sync_waits(nc)
    return nc


_cached_nc = None


def kernel(x, eps, W, qz_w1, qz_b1, qz_w2, qz_b2, zm_w, zm_b, zl_w, zl_b,
           gen_w1, gen_w2, gen_b2, col_w, col_b):
    global _cached_nc
    f32 = np.float32

    x = np.asarray(x, dtype=f32)
    eps = np.asarray(eps, dtype=f32)
    W = np.asarray(W, dtype=f32)
    qz_w1 = np.asarray(qz_w1, dtype=f32)
    qz_b1 = np.asarray(qz_b1, dtype=f32)
    qz_w2 = np.asarray(qz_w2, dtype=f32)
    qz_b2 = np.asarray(qz_b2, dtype=f32)
    zm_w = np.asarray(zm_w, dtype=f32)
    zm_b = np.asarray(zm_b, dtype=f32)
    zl_w = np.asarray(zl_w, dtype=f32)
    zl_b = np.asarray(zl_b, dtype=f32)
    gen_w1 = np.asarray(gen_w1, dtype=f32)
    gen_w2 = np.asarray(gen_w2, dtype=f32)
    gen_b2 = np.asarray(gen_b2, dtype=f32)
    col_w = np.asarray(col_w, dtype=f32)
    col_b = np.asarray(col_b, dtype=f32)

    shared = {
        "xt": np.ascontiguousarray(x.T),
        "epst": np.ascontiguousarray(eps.T),
        "qzw1": qz_w1,
        "qzb1": qz_b1.reshape(H, 1),
        "qzw2": qz_w2,
        "qzb2": qz_b2.reshape(H, 1),
        "zmw": zm_w,
        "zmb": zm_b.reshape(L, 1),
        "zlw": zl_w,
        "zlb": zl_b.reshape(L, 1),
        "hzlb": (0.5 * zl_b).reshape(L, 1),
        "w1r": np.tile(gen_w1, (4, 1)).astype(np.float16),
        "w2": gen_w2.astype(np.float16),
        "b2": gen_b2.reshape(H, 1),
    }

    in_maps = []
    for c in range(NCORES):
        sl = slice(c * DCORE, (c + 1) * DCORE)
        Wc = W[sl]                                 # [64, 32]
        w4c = np.ascontiguousarray(
            Wc.reshape(NGROUPS, 4, L).transpose(1, 2, 0).reshape(128, NGROUPS))
        cwc = np.zeros((H, DCORE * 32), dtype=np.float16)
        for j in range(DCORE):
            r = (j // 4) % 4
            cwc[:, 32 * j + r] = col_w[c * DCORE + j].astype(np.float16)
        cbc = np.zeros((128, NGROUPS // 4), dtype=f32)
        for Gs in range(NGROUPS // 4):
            for r in range(4):
                for i in range(4):
                    cbc[32 * i + r, Gs] = col_b[c * DCORE + 16 * Gs + 4 * r + i]
        m = dict(shared)
        m.update({"w4": w4c, "cwf": cwc, "cbf": cbc})
        in_maps.append(m)

    if _cached_nc is None:
        _cached_nc = _build_program()

    res = run_bass_kernel_spmd(_cached_nc, in_maps, core_ids=list(range(NCORES)))
    kernel._last_results = res

    x_mean = np.concatenate([res.results[c]["xpart"] for c in range(NCORES)], axis=0).T
    z = res.results[0]["zt_o"].T
    z_mean = res.results[0]["zmt_o"].T
    z_log_var = res.results[0]["zlvt_o"].T
    return (np.ascontiguousarray(x_mean), np.ascontiguousarray(z),
            np.ascontiguousarray(z_mean), np.ascontiguousarray(z_log_var))


# revision 9
# speedup vs baseline: 1.1894x; 1.1894x over previous
"""Trainium2 Bass kernel for nn_BaseSparseVAE (B=512, D=512, L=32, H=300).

Strategy (8 NeuronCores):
  - Shard the decoder's per-column axis D=512 across cores: 64 columns each,
    processed as 16 groups of 4 columns.
  - Encoder is tiny; replicate it on every core (no collectives needed).
  - Everything runs in transposed [feature, batch] layout so biases/masks are
    per-partition and matmuls are lhsT.T @ rhs with batch as the moving dim.
  - Matmul dtypes: encoder in float32r (accuracy for the z outputs), decoder
    in fp16 (1 cyc/row streaming + tile_position packing of the 4 columns).
  - Relu/bias work is split between the Scalar (ACT) and Vector (DVE) engines.

Self-contained: includes the walrus single-sync-wait workarounds inline.
"""

import numpy as np

import concourse.bass as bass
import concourse.mybir as mybir
from concourse.tile import TileContext
from concourse.vector_clock import ScopedClock
from concourse.bass_utils import run_bass_kernel_spmd

FP32 = mybir.dt.float32
F32R = mybir.dt.float32r
FP16 = mybir.dt.float16
AF = mybir.ActivationFunctionType
ALU = mybir.AluOpType

B, D, L, H = 512, 512, 32, 300
NCORES = 8
DCORE = D // NCORES          # 64 columns per core
NGROUPS = DCORE // 4         # 16 groups of 4 columns
MC = [0, 128, 256, 300]      # H chunk boundaries (partition chunks)
DC = [0, 128, 256, 384, 512]  # D chunk boundaries

# ---------------------------------------------------------------------------
# walrus workarounds: this toolchain accepts only ONE inline sync wait per
# instruction. Split excess waits onto same-engine EventSemaphore carriers.
# ---------------------------------------------------------------------------

def _patched_drain_and_barrier(self, tick_clock, wait_clock):
    nc = self.nc
    drain_inst = nc.sync.drain()
    wait_clock.add_sem_waits(
        drain_inst.ins, ScopedClock({None: tick_clock.global_clock})
    )
    si = drain_inst.ins.sync_info
    waits = list(si.on_wait) if si and si.on_wait else []
    if len(waits) > 1:
        del si.on_wait[1:]
        for w in waits[1:]:
            extra = nc.sync.drain()
            extra.ins.sync_info = mybir.SyncInfo(on_wait=[w], on_update=[])
    nc.all_engine_barrier()
    popped = nc._tile_sem_poison_stack.pop()
    assert popped is self._sem_poison
    nc.clear_and_free_semaphores(list(self.sems.allocated().values()))
    nc.all_engine_barrier()


TileContext._drain_and_barrier = _patched_drain_and_barrier

_carrier_ctr = [0]


def _split_sync_waits(nc, max_inline=1):
    moved = 0
    for f in nc.m.functions:
        for blk in f.blocks:
            new = []
            changed = False
            for inst in blk.instructions:
                si = inst.sync_info
                waits = list(si.on_wait) if si and si.on_wait else []
                if len(waits) > max_inline:
                    for w in waits[max_inline:]:
                        _carrier_ctr[0] += 1
                        new.append(mybir.InstEventSemaphore(
                            name=f"waitsplit_{_carrier_ctr[0]}",
                            ins=[], outs=[], engine=inst.engine,
                            sync_info=mybir.SyncInfo(on_wait=[w], on_update=[]),
                        ))
                        moved += 1
                    del si.on_wait[max_inline:]
                    changed = True
                new.append(inst)
            if changed:
                blk.instructions = new
    return moved


# ---------------------------------------------------------------------------
# program builder
# ---------------------------------------------------------------------------

def _build_program():
    nc = bass.Bass("TRN2")

    # inputs (per-core views; most are identical on all cores)
    xt = nc.dram_tensor("xt", [D, B], F32R, kind="ExternalInput")
    epst = nc.dram_tensor("epst", [L, B], FP32, kind="ExternalInput")
    qzw1 = nc.dram_tensor("qzw1", [D, H], F32R, kind="ExternalInput")
    qzb1 = nc.dram_tensor("qzb1", [H, 1], FP32, kind="ExternalInput")
    qzw2 = nc.dram_tensor("qzw2", [H, H], F32R, kind="ExternalInput")
    qzb2 = nc.dram_tensor("qzb2", [H, 1], FP32, kind="ExternalInput")
    zmw = nc.dram_tensor("zmw", [H, L], F32R, kind="ExternalInput")
    zmb = nc.dram_tensor("zmb", [L, 1], FP32, kind="ExternalInput")
    zlw = nc.dram_tensor("zlw", [H, L], F32R, kind="ExternalInput")
    zlb = nc.dram_tensor("zlb", [L, 1], FP32, kind="ExternalInput")
    hzlb = nc.dram_tensor("hzlb", [L, 1], FP32, kind="ExternalInput")  # 0.5*zl_b
    w1r = nc.dram_tensor("w1r", [128, H], FP16, kind="ExternalInput")  # gen_w1 4-rep
    w2 = nc.dram_tensor("w2", [H, H], FP16, kind="ExternalInput")
    b2 = nc.dram_tensor("b2", [H, 1], FP32, kind="ExternalInput")
    w4 = nc.dram_tensor("w4", [128, NGROUPS], FP32, kind="ExternalInput")   # per-core
    # col_w in fat zero-padded blocks: per column j a [H, 32] block whose only
    # nonzero col is r = (j//4) % 4 -> final matmuls write M=32 rows per strip,
    # accumulating 4 groups into one PSUM bank before a single copy-out.
    cwf = nc.dram_tensor("cwf", [H, DCORE * 32], FP16, kind="ExternalInput")  # per-core
    cbf = nc.dram_tensor("cbf", [128, NGROUPS // 4], FP32, kind="ExternalInput")  # per-core

    # outputs
    xpart = nc.dram_tensor("xpart", [DCORE, B], FP32, kind="ExternalOutput")
    zt_o = nc.dram_tensor("zt_o", [L, B], FP32, kind="ExternalOutput")
    zmt_o = nc.dram_tensor("zmt_o", [L, B], FP32, kind="ExternalOutput")
    zlvt_o = nc.dram_tensor("zlvt_o", [L, B], FP32, kind="ExternalOutput")

    with TileContext(nc) as tc:
        with (
            tc.tile_pool(name="wsb", bufs=1) as wsb,      # weights + persistent
            tc.tile_pool(name="hsb", bufs=1) as hsb,      # encoder activations
            tc.tile_pool(name="dec", bufs=2) as dec,      # decoder activations
        ):
            # ---- weight loads ----
            txt = []
            for ki in range(4):
                t = wsb.tile([128, B], F32R, tag=f"xt{ki}")
                nc.sync.dma_start(t[:, :], xt[DC[ki]:DC[ki + 1], :])
                txt.append(t)
            tq1 = {}
            for ki in range(4):
                for mi in range(3):
                    mc = MC[mi + 1] - MC[mi]
                    t = wsb.tile([128, mc], F32R, tag=f"q1_{ki}_{mi}")
                    nc.sync.dma_start(t[:, :], qzw1[DC[ki]:DC[ki + 1], MC[mi]:MC[mi + 1]])
                    tq1[ki, mi] = t
            tq2 = {}
            tw2 = {}
            for ki in range(3):
                kc = MC[ki + 1] - MC[ki]
                for mi in range(3):
                    mc = MC[mi + 1] - MC[mi]
                    t = wsb.tile([kc, mc], F32R, tag=f"q2_{ki}_{mi}")
                    nc.sync.dma_start(t[:, :], qzw2[MC[ki]:MC[ki + 1], MC[mi]:MC[mi + 1]])
                    tq2[ki, mi] = t
                    t = wsb.tile([kc, mc], FP16, tag=f"w2_{ki}_{mi}")
                    nc.sync.dma_start(t[:, :], w2[MC[ki]:MC[ki + 1], MC[mi]:MC[mi + 1]])
                    tw2[ki, mi] = t
            tzmw, tzlw, tcw = {}, {}, {}
            for ki in range(3):
                kc = MC[ki + 1] - MC[ki]
                t = wsb.tile([kc, L], F32R, tag=f"zmw{ki}")
                nc.sync.dma_start(t[:, :], zmw[MC[ki]:MC[ki + 1], :])
                tzmw[ki] = t
                t = wsb.tile([kc, L], F32R, tag=f"zlw{ki}")
                nc.sync.dma_start(t[:, :], zlw[MC[ki]:MC[ki + 1], :])
                tzlw[ki] = t
                t = wsb.tile([kc, DCORE * 32], FP16, tag=f"cw{ki}")
                nc.sync.dma_start(t[:, :], cwf[MC[ki]:MC[ki + 1], :])
                tcw[ki] = t
            tb1, tb2q, tb2 = {}, {}, {}
            for mi in range(3):
                mc = MC[mi + 1] - MC[mi]
                t = wsb.tile([mc, 1], FP32, tag=f"b1_{mi}")
                nc.sync.dma_start(t[:, :], qzb1[MC[mi]:MC[mi + 1], :])
                tb1[mi] = t
                t = wsb.tile([mc, 1], FP32, tag=f"b2q_{mi}")
                nc.sync.dma_start(t[:, :], qzb2[MC[mi]:MC[mi + 1], :])
                tb2q[mi] = t
                t = wsb.tile([mc, 1], FP32, tag=f"b2_{mi}")
                nc.sync.dma_start(t[:, :], b2[MC[mi]:MC[mi + 1], :])
                tb2[mi] = t
            tzmb = wsb.tile([L, 1], FP32, tag="zmb")
            nc.sync.dma_start(tzmb[:, :], zmb[:, :])
            tzlb = wsb.tile([L, 1], FP32, tag="zlb")
            nc.sync.dma_start(tzlb[:, :], zlb[:, :])
            thzlb = wsb.tile([L, 1], FP32, tag="hzlb")
            nc.sync.dma_start(thzlb[:, :], hzlb[:, :])
            teps = wsb.tile([L, B], FP32, tag="eps")
            nc.sync.dma_start(teps[:, :], epst[:, :])
            tw1 = wsb.tile([128, H], FP16, tag="w1r")
            nc.sync.dma_start(tw1[:, :], w1r[:, :])
            tw4 = wsb.tile([128, NGROUPS], FP32, tag="w4")
            nc.sync.dma_start(tw4[:, :], w4[:, :])
            tcb = wsb.tile([128, NGROUPS // 4], FP32, tag="cbf")
            nc.sync.dma_start(tcb[:, :], cbf[:, :])

            # ---- encoder ----
            with tc.tile_pool(name="eps_ps", bufs=1, space="PSUM") as eps_ps:
                h1 = {}
                for mi in range(3):
                    mc = MC[mi + 1] - MC[mi]
                    pt = eps_ps.tile([128, B], FP32, tag=f"ph1_{mi}")
                    for ki in range(4):
                        nc.tensor.matmul(pt[0:mc, :], tq1[ki, mi][:, :], txt[ki][:, :],
                                         start=(ki == 0), stop=(ki == 3))
                    t = hsb.tile([128, B], F32R, tag=f"h1_{mi}")
                    nc.scalar.activation(t[0:mc, :], pt[0:mc, :], AF.Relu,
                                         bias=tb1[mi][:, :])
                    h1[mi] = t
                h2 = {}
                for mi in range(3):
                    mc = MC[mi + 1] - MC[mi]
                    pt = eps_ps.tile([128, B], FP32, tag=f"ph2_{mi}")
                    for ki in range(3):
                        kc = MC[ki + 1] - MC[ki]
                        nc.tensor.matmul(pt[0:mc, :], tq2[ki, mi][:, :], h1[ki][0:kc, :],
                                         start=(ki == 0), stop=(ki == 2))
                    t = hsb.tile([128, B], F32R, tag=f"h2_{mi}")
                    nc.scalar.activation(t[0:mc, :], pt[0:mc, :], AF.Relu,
                                         bias=tb2q[mi][:, :])
                    h2[mi] = t
                # z heads
                pzm = eps_ps.tile([L, B], FP32, tag="pzm")
                pzl = eps_ps.tile([L, B], FP32, tag="pzl")
                for ki in range(3):
                    kc = MC[ki + 1] - MC[ki]
                    nc.tensor.matmul(pzm[:, :], tzmw[ki][:, :], h2[ki][0:kc, :],
                                     start=(ki == 0), stop=(ki == 2))
                for ki in range(3):
                    kc = MC[ki + 1] - MC[ki]
                    nc.tensor.matmul(pzl[:, :], tzlw[ki][:, :], h2[ki][0:kc, :],
                                     start=(ki == 0), stop=(ki == 2))
                tzm = hsb.tile([L, B], FP32, tag="tzm")
                nc.scalar.activation(tzm[:, :], pzm[:, :], AF.Identity, bias=tzmb[:, :])
                tzlv = hsb.tile([L, B], FP32, tag="tzlv")
                nc.scalar.activation(tzlv[:, :], pzl[:, :], AF.Identity, bias=tzlb[:, :])
                texp = hsb.tile([L, B], FP32, tag="texp")
                nc.scalar.activation(texp[:, :], pzl[:, :], AF.Exp, bias=thzlb[:, :],
                                     scale=0.5)
                # z = z_mean + eps * exp(0.5*zlv)
                tz1 = hsb.tile([L, B], FP32, tag="tz1")
                nc.vector.tensor_tensor(tz1[:, :], teps[:, :], texp[:, :],
                                        op=ALU.mult)
                nc.vector.tensor_tensor(tz1[:, :], tz1[:, :], tzm[:, :],
                                        op=ALU.add)
                # outputs
                nc.sync.dma_start(zt_o[:, :], tz1[:, :])
                nc.sync.dma_start(zmt_o[:, :], tzm[:, :])
                nc.sync.dma_start(zlvt_o[:, :], tzlv[:, :])
                # z4: 4 stacked copies of zT
                tz4 = hsb.tile([128, B], FP32, tag="tz4")
                for i in range(4):
                    nc.sync.dma_start(tz4[32 * i:32 * i + 32, :], tz1[:, :])

            # ---- decoder ----
            def relu_split(dst, src, eng, bias=None):
                if eng == 0:
                    if bias is None:
                        nc.scalar.activation(dst, src, AF.Relu)
                    else:
                        nc.scalar.activation(dst, src, AF.Relu, bias=bias)
                else:
                    if bias is None:
                        nc.vector.tensor_scalar_max(dst, src, 0.0)
                    else:
                        nc.vector.tensor_scalar(dst, src, bias, 0.0,
                                                op0=ALU.add, op1=ALU.max)

            with (
                tc.tile_pool(name="dps", bufs=1, space="PSUM") as dps,
                tc.tile_pool(name="fps", bufs=1, space="PSUM") as fps,
            ):
                for G in range(NGROUPS // 4):
                    finb = [fps.tile([128, B], FP32, tag=f"fin{i}", name=f"fin_{G}_{i}")
                            for i in range(4)]
                    for gs in range(4):
                        g = 4 * G + gs
                        msk = dec.tile([128, B], FP16, tag="msk")
                        nc.vector.tensor_scalar_mul(msk[:, :], tz4[:, :], tw4[:, g:g + 1])

                        # L1: 4-row-packed fp16 (K=32 each); g1 free-blocked by mi
                        g1 = [dec.tile([128, 3 * B], FP16, tag=f"g1_{i}", name=f"g1t_{g}_{i}")
                              for i in range(4)]
                        for mi in range(3):
                            mc = MC[mi + 1] - MC[mi]
                            pts = []
                            for i in range(4):
                                pt = dps.tile([128, B], FP32, tag=f"w{i}")
                                kw = {}
                                if i == 3:
                                    kw["tile_position"] = (96, 0)
                                nc.tensor.matmul(pt[0:mc, :],
                                                 tw1[32 * i:32 * i + 32, MC[mi]:MC[mi + 1]],
                                                 msk[32 * i:32 * i + 32, :],
                                                 start=True, stop=True, **kw)
                                pts.append(pt)
                            for i in range(4):
                                relu_split(g1[i][0:mc, B * mi:B * mi + B], pts[i][0:mc, :],
                                           (i + mi) % 2)

                        # L2: fp16, weights shared across the 4 columns (i innermost)
                        g2 = [dec.tile([128, 3 * B], FP16, tag=f"g2_{i}", name=f"g2t_{g}_{i}")
                              for i in range(4)]
                        for mi in range(3):
                            mc = MC[mi + 1] - MC[mi]
                            pts = [dps.tile([128, B], FP32, tag=f"w{i}", name=f"l2p_{g}_{mi}_{i}")
                                   for i in range(4)]
                            for ki in range(3):
                                kc = MC[ki + 1] - MC[ki]
                                for i in range(4):
                                    nc.tensor.matmul(pts[i][0:mc, :], tw2[ki, mi][:, :],
                                                     g1[i][0:kc, B * ki:B * ki + B],
                                                     start=(ki == 0), stop=(ki == 2))
                            for i in range(4):
                                relu_split(g2[i][0:mc, B * mi:B * mi + B], pts[i][0:mc, :],
                                           (i + mi) % 2, bias=tb2[mi][:, :])

                        # final: fp16 matvecs, strip i -> own bank, M=32 rows with
                        # zero-padded lhsT; 4 groups accumulate before copy-out
                        for ki in range(3):
                            kc = MC[ki + 1] - MC[ki]
                            for i in range(4):
                                jloc = 4 * g + i
                                kw = {}
                                if i == 3:
                                    kw["tile_position"] = (0, 96)
                                nc.tensor.matmul(finb[i][32 * i:32 * i + 32, :],
                                                 tcw[ki][:, 32 * jloc:32 * jloc + 32],
                                                 g2[i][0:kc, B * ki:B * ki + B],
                                                 start=(gs == 0 and ki == 0),
                                                 stop=(gs == 3 and ki == 2), **kw)

                    # copy out the 4 accumulated banks (+ col_b) and store
                    xr = dec.tile([128, B], FP32, tag="xr", name=f"xr_{G}")
                    for i in range(4):
                        nc.vector.tensor_scalar_add(
                            xr[32 * i:32 * i + 4, :], finb[i][32 * i:32 * i + 4, :],
                            tcb[32 * i:32 * i + 4, G:G + 1])
                        nc.sync.dma_start(xpart[16 * G + i:16 * G + 16:4, :],
                                          xr[32 * i:32 * i + 4, :])

    _split_sync_waits(nc)
    return nc


_cached_nc = None


def kernel(x, eps, W, qz_w1, qz_b1, qz_w2, qz_b2, zm_w, zm_b, zl_w, zl_b,
           gen_w1, gen_w2, gen_b2, col_w, col_b):
    global _cached_nc
    f32 = np.float32

    x = np.asarray(x, dtype=f32)
    eps = np.asarray(eps, dtype=f32)
    W = np.asarray(W, dtype=f32)
    qz_w1 = np.asarray(qz_w1, dtype=f32)
    qz_b1 = np.asarray(qz_b1, dtype=f32)
    qz_w2 = np.asarray(qz_w2, dtype=f32)
    qz_b2 = np.asarray(qz_b2, dtype=f32)
    zm_w = np.asarray(zm_w, dtype=f32)
    zm_b = np.asarray(zm_b, dtype=f32)
    zl_w = np.asarray(zl_w, dtype=f32)
    zl_b = np.asarray(zl_b, dtype=f32)
    gen_w1 = np.asarray(gen_w1, dtype=f32)
    gen_w2 = np.asarray(gen_w2, dtype=f32)
    gen_b2 = np.asarray(gen_b2, dtype=f32)
    col_w = np.asarray(col_w, dtype=f32)
    col_b = np.asarray(col_b, dtype=f32)

    shared = {
        "xt": np.ascontiguousarray(x.T),
        "epst": np.ascontiguousarray(eps.T),
        "qzw1": qz_w1,
        "qzb1": qz_b1.reshape(H, 1),
        "qzw2": qz_w2,
        "qzb2": qz_b2.reshape(H, 1),
        "zmw": zm_w,
        "zmb": zm_b.reshape(L, 1),
        "zlw": zl_w,
        "zlb": zl_b.reshape(L, 1),
        "hzlb": (0.5 * zl_b).reshape(L, 1),
        "w1r": np.tile(gen_w1, (4, 1)).astype(np.float16),
        "w2": gen_w2.astype(np.float16),
        "b2": gen_b2.reshape(H, 1),
    }

    in_maps = []
    for c in range(NCORES):
        sl = slice(c * DCORE, (c + 1) * DCORE)
        Wc = W[sl]                                 # [64, 32]
        w4c = np.ascontiguousarray(
            Wc.reshape(NGROUPS, 4, L).transpose(1, 2, 0).reshape(128, NGROUPS))
        cwc = np.zeros((H, DCORE * 32), dtype=np.float16)
        for j in range(DCORE):
            r = (j // 4) % 4
            cwc[:, 32 * j + r] = col_w[c * DCORE + j].astype(np.float16)
        cbc = np.zeros((128, NGROUPS // 4), dtype=f32)
        for Gs in range(NGROUPS // 4):
            for r in range(4):
                for i in range(4):
                    cbc[32 * i + r, Gs] = col_b[c * DCORE + 16 * Gs + 4 * r + i]
        m = dict(shared)
        m.update({"w4": w4c, "cwf": cwc, "cbf": cbc})
        in_maps.append(m)

    if _cached_nc is None:
        _cached_nc = _build_program()

    res = run_bass_kernel_spmd(_cached_nc, in_maps, core_ids=list(range(NCORES)))
    kernel._last_results = res

    x_mean = np.concatenate([res.results[c]["xpart"] for c in range(NCORES)], axis=0).T
    z = res.results[0]["zt_o"].T
    z_mean = res.results[0]["zmt_o"].T
    z_log_var = res.results[0]["zlvt_o"].T
    return (np.ascontiguousarray(x_mean), np.ascontiguousarray(z),
            np.ascontiguousarray(z_mean), np.ascontiguousarray(z_log_var))


# revision 10
# speedup vs baseline: 1.2427x; 1.0448x over previous
"""Trainium2 Bass kernel for nn_BaseSparseVAE (B=512, D=512, L=32, H=300).

Strategy (8 NeuronCores):
  - Shard the decoder's per-column axis D=512 across cores: 64 columns each,
    processed as 16 groups of 4 columns.
  - Encoder is tiny; replicate it on every core (no collectives needed).
  - Everything runs in transposed [feature, batch] layout so biases/masks are
    per-partition and matmuls are lhsT.T @ rhs with batch as the moving dim.
  - Matmul dtypes: encoder in float32r (accuracy for the z outputs), decoder
    in fp16 (1 cyc/row streaming + tile_position packing of the 4 columns).
  - Relu/bias work is split between the Scalar (ACT) and Vector (DVE) engines.

Self-contained: includes the walrus single-sync-wait workarounds inline.
"""

import numpy as np

import concourse.bass as bass
import concourse.mybir as mybir
from concourse.tile import TileContext
from concourse.vector_clock import ScopedClock
from concourse.bass_utils import run_bass_kernel_spmd

FP32 = mybir.dt.float32
F32R = mybir.dt.float32r
FP16 = mybir.dt.float16
AF = mybir.ActivationFunctionType
ALU = mybir.AluOpType

B, D, L, H = 512, 512, 32, 300
NCORES = 8
DCORE = D // NCORES          # 64 columns per core
NGROUPS = DCORE // 4         # 16 groups of 4 columns
MC = [0, 128, 256, 300]      # H chunk boundaries (partition chunks)
DC = [0, 128, 256, 384, 512]  # D chunk boundaries

# ---------------------------------------------------------------------------
# walrus workarounds: this toolchain accepts only ONE inline sync wait per
# instruction. Split excess waits onto same-engine EventSemaphore carriers.
# ---------------------------------------------------------------------------

def _patched_drain_and_barrier(self, tick_clock, wait_clock):
    nc = self.nc
    drain_inst = nc.sync.drain()
    wait_clock.add_sem_waits(
        drain_inst.ins, ScopedClock({None: tick_clock.global_clock})
    )
    si = drain_inst.ins.sync_info
    waits = list(si.on_wait) if si and si.on_wait else []
    if len(waits) > 1:
        del si.on_wait[1:]
        for w in waits[1:]:
            extra = nc.sync.drain()
            extra.ins.sync_info = mybir.SyncInfo(on_wait=[w], on_update=[])
    nc.all_engine_barrier()
    popped = nc._tile_sem_poison_stack.pop()
    assert popped is self._sem_poison
    nc.clear_and_free_semaphores(list(self.sems.allocated().values()))
    nc.all_engine_barrier()


TileContext._drain_and_barrier = _patched_drain_and_barrier

_carrier_ctr = [0]


def _split_sync_waits(nc, max_inline=1):
    moved = 0
    for f in nc.m.functions:
        for blk in f.blocks:
            new = []
            changed = False
            for inst in blk.instructions:
                si = inst.sync_info
                waits = list(si.on_wait) if si and si.on_wait else []
                if len(waits) > max_inline:
                    for w in waits[max_inline:]:
                        _carrier_ctr[0] += 1
                        new.append(mybir.InstEventSemaphore(
                            name=f"waitsplit_{_carrier_ctr[0]}",
                            ins=[], outs=[], engine=inst.engine,
                            sync_info=mybir.SyncInfo(on_wait=[w], on_update=[]),
                        ))
                        moved += 1
                    del si.on_wait[max_inline:]
                    changed = True
                new.append(inst)
            if changed:
                blk.instructions = new
    return moved


# ---------------------------------------------------------------------------
# program builder
# ---------------------------------------------------------------------------

def _build_program():
    nc = bass.Bass("TRN2")

    # inputs (per-core views; most are identical on all cores)
    xt = nc.dram_tensor("xt", [D, B], FP16, kind="ExternalInput")
    epst = nc.dram_tensor("epst", [L, B], FP32, kind="ExternalInput")
    qzw1 = nc.dram_tensor("qzw1", [D, H], FP16, kind="ExternalInput")
    qzb1 = nc.dram_tensor("qzb1", [H, 1], FP32, kind="ExternalInput")
    qzw2 = nc.dram_tensor("qzw2", [H, H], F32R, kind="ExternalInput")
    qzb2 = nc.dram_tensor("qzb2", [H, 1], FP32, kind="ExternalInput")
    zmw = nc.dram_tensor("zmw", [H, L], F32R, kind="ExternalInput")
    zmb = nc.dram_tensor("zmb", [L, 1], FP32, kind="ExternalInput")
    zlw = nc.dram_tensor("zlw", [H, L], F32R, kind="ExternalInput")
    zlb = nc.dram_tensor("zlb", [L, 1], FP32, kind="ExternalInput")
    hzlb = nc.dram_tensor("hzlb", [L, 1], FP32, kind="ExternalInput")  # 0.5*zl_b
    w1r = nc.dram_tensor("w1r", [128, H], FP16, kind="ExternalInput")  # gen_w1 4-rep
    w2 = nc.dram_tensor("w2", [H, H], FP16, kind="ExternalInput")
    b2 = nc.dram_tensor("b2", [H, 1], FP32, kind="ExternalInput")
    w4 = nc.dram_tensor("w4", [128, NGROUPS], FP32, kind="ExternalInput")   # per-core
    # col_w in fat zero-padded blocks: per column j a [H, 32] block whose only
    # nonzero col is r = (j//4) % 4 -> final matmuls write M=32 rows per strip,
    # accumulating 4 groups into one PSUM bank before a single copy-out.
    cwf = nc.dram_tensor("cwf", [H, DCORE * 32], FP16, kind="ExternalInput")  # per-core
    cbf = nc.dram_tensor("cbf", [128, NGROUPS // 4], FP32, kind="ExternalInput")  # per-core

    # outputs
    xpart = nc.dram_tensor("xpart", [DCORE, B], FP32, kind="ExternalOutput")
    zt_o = nc.dram_tensor("zt_o", [L, B], FP32, kind="ExternalOutput")
    zmt_o = nc.dram_tensor("zmt_o", [L, B], FP32, kind="ExternalOutput")
    zlvt_o = nc.dram_tensor("zlvt_o", [L, B], FP32, kind="ExternalOutput")

    with TileContext(nc) as tc:
        with (
            tc.tile_pool(name="wsb", bufs=1) as wsb,      # weights + persistent
            tc.tile_pool(name="hsb", bufs=1) as hsb,      # encoder activations
            tc.tile_pool(name="dec", bufs=2) as dec,      # decoder activations
        ):
            # ---- weight loads ----
            txt = []
            for ki in range(4):
                t = wsb.tile([128, B], FP16, tag=f"xt{ki}")
                nc.sync.dma_start(t[:, :], xt[DC[ki]:DC[ki + 1], :])
                txt.append(t)
            tq1 = {}
            for ki in range(4):
                for mi in range(3):
                    mc = MC[mi + 1] - MC[mi]
                    t = wsb.tile([128, mc], FP16, tag=f"q1_{ki}_{mi}")
                    nc.sync.dma_start(t[:, :], qzw1[DC[ki]:DC[ki + 1], MC[mi]:MC[mi + 1]])
                    tq1[ki, mi] = t
            tq2 = {}
            tw2 = {}
            for ki in range(3):
                kc = MC[ki + 1] - MC[ki]
                for mi in range(3):
                    mc = MC[mi + 1] - MC[mi]
                    t = wsb.tile([kc, mc], F32R, tag=f"q2_{ki}_{mi}")
                    nc.gpsimd.dma_start(t[:, :], qzw2[MC[ki]:MC[ki + 1], MC[mi]:MC[mi + 1]])
                    tq2[ki, mi] = t
                    t = wsb.tile([kc, mc], FP16, tag=f"w2_{ki}_{mi}")
                    nc.gpsimd.dma_start(t[:, :], w2[MC[ki]:MC[ki + 1], MC[mi]:MC[mi + 1]])
                    tw2[ki, mi] = t
            tzmw, tzlw, tcw = {}, {}, {}
            for ki in range(3):
                kc = MC[ki + 1] - MC[ki]
                t = wsb.tile([kc, L], F32R, tag=f"zmw{ki}")
                nc.gpsimd.dma_start(t[:, :], zmw[MC[ki]:MC[ki + 1], :])
                tzmw[ki] = t
                t = wsb.tile([kc, L], F32R, tag=f"zlw{ki}")
                nc.gpsimd.dma_start(t[:, :], zlw[MC[ki]:MC[ki + 1], :])
                tzlw[ki] = t
                t = wsb.tile([kc, DCORE * 32], FP16, tag=f"cw{ki}")
                nc.gpsimd.dma_start(t[:, :], cwf[MC[ki]:MC[ki + 1], :])
                tcw[ki] = t
            tb1, tb2q, tb2 = {}, {}, {}
            for mi in range(3):
                mc = MC[mi + 1] - MC[mi]
                t = wsb.tile([mc, 1], FP32, tag=f"b1_{mi}")
                nc.gpsimd.dma_start(t[:, :], qzb1[MC[mi]:MC[mi + 1], :])
                tb1[mi] = t
                t = wsb.tile([mc, 1], FP32, tag=f"b2q_{mi}")
                nc.gpsimd.dma_start(t[:, :], qzb2[MC[mi]:MC[mi + 1], :])
                tb2q[mi] = t
                t = wsb.tile([mc, 1], FP32, tag=f"b2_{mi}")
                nc.gpsimd.dma_start(t[:, :], b2[MC[mi]:MC[mi + 1], :])
                tb2[mi] = t
            tzmb = wsb.tile([L, 1], FP32, tag="zmb")
            nc.gpsimd.dma_start(tzmb[:, :], zmb[:, :])
            tzlb = wsb.tile([L, 1], FP32, tag="zlb")
            nc.gpsimd.dma_start(tzlb[:, :], zlb[:, :])
            thzlb = wsb.tile([L, 1], FP32, tag="hzlb")
            nc.gpsimd.dma_start(thzlb[:, :], hzlb[:, :])
            teps = wsb.tile([L, B], FP32, tag="eps")
            nc.gpsimd.dma_start(teps[:, :], epst[:, :])
            tw1 = wsb.tile([128, H], FP16, tag="w1r")
            nc.gpsimd.dma_start(tw1[:, :], w1r[:, :])
            tw4 = wsb.tile([128, NGROUPS], FP32, tag="w4")
            nc.gpsimd.dma_start(tw4[:, :], w4[:, :])
            tcb = wsb.tile([128, NGROUPS // 4], FP32, tag="cbf")
            nc.gpsimd.dma_start(tcb[:, :], cbf[:, :])

            # ---- encoder ----
            with tc.tile_pool(name="eps_ps", bufs=1, space="PSUM") as eps_ps:
                h1 = {}
                for mi in range(3):
                    mc = MC[mi + 1] - MC[mi]
                    pt = eps_ps.tile([128, B], FP32, tag=f"ph1_{mi}")
                    for ki in range(4):
                        nc.tensor.matmul(pt[0:mc, :], tq1[ki, mi][:, :], txt[ki][:, :],
                                         start=(ki == 0), stop=(ki == 3))
                    t = hsb.tile([128, B], F32R, tag=f"h1_{mi}")
                    nc.scalar.activation(t[0:mc, :], pt[0:mc, :], AF.Relu,
                                         bias=tb1[mi][:, :])
                    h1[mi] = t
                h2 = {}
                for mi in range(3):
                    mc = MC[mi + 1] - MC[mi]
                    pt = eps_ps.tile([128, B], FP32, tag=f"ph2_{mi}")
                    for ki in range(3):
                        kc = MC[ki + 1] - MC[ki]
                        nc.tensor.matmul(pt[0:mc, :], tq2[ki, mi][:, :], h1[ki][0:kc, :],
                                         start=(ki == 0), stop=(ki == 2))
                    t = hsb.tile([128, B], F32R, tag=f"h2_{mi}")
                    nc.scalar.activation(t[0:mc, :], pt[0:mc, :], AF.Relu,
                                         bias=tb2q[mi][:, :])
                    h2[mi] = t
                # z heads
                pzm = eps_ps.tile([L, B], FP32, tag="pzm")
                pzl = eps_ps.tile([L, B], FP32, tag="pzl")
                for ki in range(3):
                    kc = MC[ki + 1] - MC[ki]
                    nc.tensor.matmul(pzm[:, :], tzmw[ki][:, :], h2[ki][0:kc, :],
                                     start=(ki == 0), stop=(ki == 2))
                for ki in range(3):
                    kc = MC[ki + 1] - MC[ki]
                    nc.tensor.matmul(pzl[:, :], tzlw[ki][:, :], h2[ki][0:kc, :],
                                     start=(ki == 0), stop=(ki == 2))
                tzm = hsb.tile([L, B], FP32, tag="tzm")
                nc.scalar.activation(tzm[:, :], pzm[:, :], AF.Identity, bias=tzmb[:, :])
                tzlv = hsb.tile([L, B], FP32, tag="tzlv")
                nc.scalar.activation(tzlv[:, :], pzl[:, :], AF.Identity, bias=tzlb[:, :])
                texp = hsb.tile([L, B], FP32, tag="texp")
                nc.scalar.activation(texp[:, :], pzl[:, :], AF.Exp, bias=thzlb[:, :],
                                     scale=0.5)
                # z = z_mean + eps * exp(0.5*zlv)
                tz1 = hsb.tile([L, B], FP32, tag="tz1")
                nc.vector.tensor_tensor(tz1[:, :], teps[:, :], texp[:, :],
                                        op=ALU.mult)
                nc.vector.tensor_tensor(tz1[:, :], tz1[:, :], tzm[:, :],
                                        op=ALU.add)
                # outputs
                nc.sync.dma_start(zt_o[:, :], tz1[:, :])
                nc.sync.dma_start(zmt_o[:, :], tzm[:, :])
                nc.sync.dma_start(zlvt_o[:, :], tzlv[:, :])
                # z4: 4 stacked copies of zT
                tz4 = hsb.tile([128, B], FP32, tag="tz4")
                for i in range(4):
                    nc.sync.dma_start(tz4[32 * i:32 * i + 32, :], tz1[:, :])

            # ---- decoder ----
            def relu_split(dst, src, eng, bias=None):
                if eng == 0:
                    if bias is None:
                        nc.scalar.activation(dst, src, AF.Relu)
                    else:
                        nc.scalar.activation(dst, src, AF.Relu, bias=bias)
                else:
                    if bias is None:
                        nc.vector.tensor_scalar_max(dst, src, 0.0)
                    else:
                        nc.vector.tensor_scalar(dst, src, bias, 0.0,
                                                op0=ALU.add, op1=ALU.max)

            with (
                tc.tile_pool(name="dps", bufs=1, space="PSUM") as dps,
                tc.tile_pool(name="fps", bufs=1, space="PSUM") as fps,
            ):
                def make_msk(g):
                    m = dec.tile([128, B], FP16, tag="msk", name=f"msk_{g}")
                    nc.vector.tensor_scalar_mul(m[:, :], tz4[:, :], tw4[:, g:g + 1])
                    return m

                msk = make_msk(0)
                for G in range(NGROUPS // 4):
                    finb = [fps.tile([128, B], FP32, tag=f"fin{i}", name=f"fin_{G}_{i}")
                            for i in range(4)]
                    for gs in range(4):
                        g = 4 * G + gs

                        # L1: 4-row-packed fp16 (K=32 each); g1 free-blocked by mi
                        g1 = [dec.tile([128, 3 * B], FP16, tag=f"g1_{i}", name=f"g1t_{g}_{i}")
                              for i in range(4)]
                        for mi in range(3):
                            mc = MC[mi + 1] - MC[mi]
                            pts = []
                            for i in range(4):
                                pt = dps.tile([128, B], FP32, tag=f"w{i}")
                                kw = {}
                                if i == 3:
                                    kw["tile_position"] = (96, 0)
                                nc.tensor.matmul(pt[0:mc, :],
                                                 tw1[32 * i:32 * i + 32, MC[mi]:MC[mi + 1]],
                                                 msk[32 * i:32 * i + 32, :],
                                                 start=True, stop=True, **kw)
                                pts.append(pt)
                            for i in range(4):
                                relu_split(g1[i][0:mc, B * mi:B * mi + B], pts[i][0:mc, :],
                                           (i + mi) % 2)

                        if g + 1 < NGROUPS:
                            nxt_msk = make_msk(g + 1)

                        # L2: fp16, weights shared across the 4 columns (i innermost)
                        g2 = [dec.tile([128, 3 * B], FP16, tag=f"g2_{i}", name=f"g2t_{g}_{i}")
                              for i in range(4)]
                        for mi in range(3):
                            mc = MC[mi + 1] - MC[mi]
                            pts = [dps.tile([128, B], FP32, tag=f"w{i}", name=f"l2p_{g}_{mi}_{i}")
                                   for i in range(4)]
                            for ki in range(3):
                                kc = MC[ki + 1] - MC[ki]
                                for i in range(4):
                                    nc.tensor.matmul(pts[i][0:mc, :], tw2[ki, mi][:, :],
                                                     g1[i][0:kc, B * ki:B * ki + B],
                                                     start=(ki == 0), stop=(ki == 2))
                            for i in range(4):
                                relu_split(g2[i][0:mc, B * mi:B * mi + B], pts[i][0:mc, :],
                                           (i + mi) % 2, bias=tb2[mi][:, :])

                        # final: fp16 matvecs, strip i -> own bank, M=32 rows with
                        # zero-padded lhsT; 4 groups accumulate before copy-out
                        for ki in range(3):
                            kc = MC[ki + 1] - MC[ki]
                            for i in range(4):
                                jloc = 4 * g + i
                                kw = {}
                                if i == 3:
                                    kw["tile_position"] = (0, 96)
                                nc.tensor.matmul(finb[i][32 * i:32 * i + 32, :],
                                                 tcw[ki][:, 32 * jloc:32 * jloc + 32],
                                                 g2[i][0:kc, B * ki:B * ki + B],
                                                 start=(gs == 0 and ki == 0),
                                                 stop=(gs == 3 and ki == 2), **kw)

                        if g + 1 < NGROUPS:
                            msk = nxt_msk

                    # copy out the 4 accumulated banks (+ col_b) and store
                    xr = dec.tile([128, B], FP32, tag="xr", name=f"xr_{G}")
                    for i in range(4):
                        nc.vector.tensor_scalar_add(
                            xr[32 * i:32 * i + 4, :], finb[i][32 * i:32 * i + 4, :],
                            tcb[32 * i:32 * i + 4, G:G + 1])
                        nc.sync.dma_start(xpart[16 * G + i:16 * G + 16:4, :],
                                          xr[32 * i:32 * i + 4, :])

    _split_sync_waits(nc)
    return nc


_cached_nc = None


def kernel(x, eps, W, qz_w1, qz_b1, qz_w2, qz_b2, zm_w, zm_b, zl_w, zl_b,
           gen_w1, gen_w2, gen_b2, col_w, col_b):
    global _cached_nc
    f32 = np.float32

    x = np.asarray(x, dtype=f32)
    eps = np.asarray(eps, dtype=f32)
    W = np.asarray(W, dtype=f32)
    qz_w1 = np.asarray(qz_w1, dtype=f32)
    qz_b1 = np.asarray(qz_b1, dtype=f32)
    qz_w2 = np.asarray(qz_w2, dtype=f32)
    qz_b2 = np.asarray(qz_b2, dtype=f32)
    zm_w = np.asarray(zm_w, dtype=f32)
    zm_b = np.asarray(zm_b, dtype=f32)
    zl_w = np.asarray(zl_w, dtype=f32)
    zl_b = np.asarray(zl_b, dtype=f32)
    gen_w1 = np.asarray(gen_w1, dtype=f32)
    gen_w2 = np.asarray(gen_w2, dtype=f32)
    gen_b2 = np.asarray(gen_b2, dtype=f32)
    col_w = np.asarray(col_w, dtype=f32)
    col_b = np.asarray(col_b, dtype=f32)

    shared = {
        "xt": np.ascontiguousarray(x.T).astype(np.float16),
        "epst": np.ascontiguousarray(eps.T),
        "qzw1": qz_w1.astype(np.float16),
        "qzb1": qz_b1.reshape(H, 1),
        "qzw2": qz_w2,
        "qzb2": qz_b2.reshape(H, 1),
        "zmw": zm_w,
        "zmb": zm_b.reshape(L, 1),
        "zlw": zl_w,
        "zlb": zl_b.reshape(L, 1),
        "hzlb": (0.5 * zl_b).reshape(L, 1),
        "w1r": np.tile(gen_w1, (4, 1)).astype(np.float16),
        "w2": gen_w2.astype(np.float16),
        "b2": gen_b2.reshape(H, 1),
    }

    in_maps = []
    for c in range(NCORES):
        sl = slice(c * DCORE, (c + 1) * DCORE)
        Wc = W[sl]                                 # [64, 32]
        w4c = np.ascontiguousarray(
            Wc.reshape(NGROUPS, 4, L).transpose(1, 2, 0).reshape(128, NGROUPS))
        cwc = np.zeros((H, DCORE * 32), dtype=np.float16)
        for j in range(DCORE):
            r = (j // 4) % 4
            cwc[:, 32 * j + r] = col_w[c * DCORE + j].astype(np.float16)
        cbc = np.zeros((128, NGROUPS // 4), dtype=f32)
        for Gs in range(NGROUPS // 4):
            for r in range(4):
                for i in range(4):
                    cbc[32 * i + r, Gs] = col_b[c * DCORE + 16 * Gs + 4 * r + i]
        m = dict(shared)
        m.update({"w4": w4c, "cwf": cwc, "cbf": cbc})
        in_maps.append(m)

    if _cached_nc is None:
        _cached_nc = _build_program()

    res = run_bass_kernel_spmd(_cached_nc, in_maps, core_ids=list(range(NCORES)))
    kernel._last_results = res

    x_mean = np.concatenate([res.results[c]["xpart"] for c in range(NCORES)], axis=0).T
    z = res.results[0]["zt_o"].T
    z_mean = res.results[0]["zmt_o"].T
    z_log_var = res.results[0]["zlvt_o"].T
    return (np.ascontiguousarray(x_mean), np.ascontiguousarray(z),
            np.ascontiguousarray(z_mean), np.ascontiguousarray(z_log_var))


# revision 11
# speedup vs baseline: 1.2995x; 1.0457x over previous
"""Trainium2 Bass kernel for nn_BaseSparseVAE (B=512, D=512, L=32, H=300).

Strategy (8 NeuronCores):
  - Shard the decoder's per-column axis D=512 across cores: 64 columns each,
    processed as 16 groups of 4 columns.
  - Encoder is tiny; replicate it on every core (no collectives needed).
  - Everything runs in transposed [feature, batch] layout so biases/masks are
    per-partition and matmuls are lhsT.T @ rhs with batch as the moving dim.
  - Matmul dtypes: encoder in float32r (accuracy for the z outputs), decoder
    in fp16 (1 cyc/row streaming + tile_position packing of the 4 columns).
  - Relu/bias work is split between the Scalar (ACT) and Vector (DVE) engines.

Self-contained: includes the walrus single-sync-wait workarounds inline.
"""

import numpy as np

import concourse.bass as bass
import concourse.mybir as mybir
from concourse.tile import TileContext
from concourse.vector_clock import ScopedClock
from concourse.bass_utils import run_bass_kernel_spmd

FP32 = mybir.dt.float32
F32R = mybir.dt.float32r
FP16 = mybir.dt.float16
AF = mybir.ActivationFunctionType
ALU = mybir.AluOpType

B, D, L, H = 512, 512, 32, 300
NCORES = 8
DCORE = D // NCORES          # 64 columns per core
NGROUPS = DCORE // 4         # 16 groups of 4 columns
MC = [0, 128, 256, 300]      # H chunk boundaries (partition chunks)
DC = [0, 128, 256, 384, 512]  # D chunk boundaries

# ---------------------------------------------------------------------------
# walrus workarounds: this toolchain accepts only ONE inline sync wait per
# instruction. Split excess waits onto same-engine EventSemaphore carriers.
# ---------------------------------------------------------------------------

def _patched_drain_and_barrier(self, tick_clock, wait_clock):
    nc = self.nc
    drain_inst = nc.sync.drain()
    wait_clock.add_sem_waits(
        drain_inst.ins, ScopedClock({None: tick_clock.global_clock})
    )
    si = drain_inst.ins.sync_info
    waits = list(si.on_wait) if si and si.on_wait else []
    if len(waits) > 1:
        del si.on_wait[1:]
        for w in waits[1:]:
            extra = nc.sync.drain()
            extra.ins.sync_info = mybir.SyncInfo(on_wait=[w], on_update=[])
    nc.all_engine_barrier()
    popped = nc._tile_sem_poison_stack.pop()
    assert popped is self._sem_poison
    nc.clear_and_free_semaphores(list(self.sems.allocated().values()))
    nc.all_engine_barrier()


TileContext._drain_and_barrier = _patched_drain_and_barrier

_carrier_ctr = [0]


def _split_sync_waits(nc, max_inline=1):
    moved = 0
    for f in nc.m.functions:
        for blk in f.blocks:
            new = []
            changed = False
            for inst in blk.instructions:
                si = inst.sync_info
                waits = list(si.on_wait) if si and si.on_wait else []
                if len(waits) > max_inline:
                    for w in waits[max_inline:]:
                        _carrier_ctr[0] += 1
                        new.append(mybir.InstEventSemaphore(
                            name=f"waitsplit_{_carrier_ctr[0]}",
                            ins=[], outs=[], engine=inst.engine,
                            sync_info=mybir.SyncInfo(on_wait=[w], on_update=[]),
                        ))
                        moved += 1
                    del si.on_wait[max_inline:]
                    changed = True
                new.append(inst)
            if changed:
                blk.instructions = new
    return moved


# ---------------------------------------------------------------------------
# program builder
# ---------------------------------------------------------------------------

def _build_program():
    nc = bass.Bass("TRN2")

    # inputs (per-core views; most are identical on all cores)
    xt = nc.dram_tensor("xt", [D, B], FP16, kind="ExternalInput")
    epst = nc.dram_tensor("epst", [L, B], FP32, kind="ExternalInput")
    qzw1 = nc.dram_tensor("qzw1", [D, H], FP16, kind="ExternalInput")
    qzb1 = nc.dram_tensor("qzb1", [H, 1], FP32, kind="ExternalInput")
    qzw2 = nc.dram_tensor("qzw2", [H, H], F32R, kind="ExternalInput")
    qzb2 = nc.dram_tensor("qzb2", [H, 1], FP32, kind="ExternalInput")
    zmw = nc.dram_tensor("zmw", [H, L], F32R, kind="ExternalInput")
    zmb = nc.dram_tensor("zmb", [L, 1], FP32, kind="ExternalInput")
    zlw = nc.dram_tensor("zlw", [H, L], F32R, kind="ExternalInput")
    zlb = nc.dram_tensor("zlb", [L, 1], FP32, kind="ExternalInput")
    hzlb = nc.dram_tensor("hzlb", [L, 1], FP32, kind="ExternalInput")  # 0.5*zl_b
    w1r = nc.dram_tensor("w1r", [128, H], FP16, kind="ExternalInput")  # gen_w1 4-rep
    w2 = nc.dram_tensor("w2", [H, H], FP16, kind="ExternalInput")
    b2 = nc.dram_tensor("b2", [H, 1], FP32, kind="ExternalInput")
    w4 = nc.dram_tensor("w4", [128, NGROUPS], FP32, kind="ExternalInput")   # per-core
    # col_w in fat zero-padded blocks: per column j a [H, 32] block whose only
    # nonzero col is r = (j//4) % 4 -> final matmuls write M=32 rows per strip,
    # accumulating 4 groups into one PSUM bank before a single copy-out.
    cwf = nc.dram_tensor("cwf", [H, DCORE * 32], FP16, kind="ExternalInput")  # per-core
    cbf = nc.dram_tensor("cbf", [128, NGROUPS // 4], FP32, kind="ExternalInput")  # per-core

    # outputs
    xpart = nc.dram_tensor("xpart", [DCORE, B], FP32, kind="ExternalOutput")
    zt_o = nc.dram_tensor("zt_o", [L, B], FP32, kind="ExternalOutput")
    zmt_o = nc.dram_tensor("zmt_o", [L, B], FP32, kind="ExternalOutput")
    zlvt_o = nc.dram_tensor("zlvt_o", [L, B], FP32, kind="ExternalOutput")

    with TileContext(nc) as tc:
        with (
            tc.tile_pool(name="wsb", bufs=1) as wsb,      # weights + persistent
            tc.tile_pool(name="hsb", bufs=1) as hsb,      # encoder activations
            tc.tile_pool(name="dec", bufs=2) as dec,      # decoder activations
        ):
            # ---- weight loads ----
            txt = []
            for ki in range(4):
                t = wsb.tile([128, B], FP16, tag=f"xt{ki}")
                nc.sync.dma_start(t[:, :], xt[DC[ki]:DC[ki + 1], :])
                txt.append(t)
            tq1 = {}
            for ki in range(4):
                for mi in range(3):
                    mc = MC[mi + 1] - MC[mi]
                    t = wsb.tile([128, mc], FP16, tag=f"q1_{ki}_{mi}")
                    nc.sync.dma_start(t[:, :], qzw1[DC[ki]:DC[ki + 1], MC[mi]:MC[mi + 1]])
                    tq1[ki, mi] = t
            tq2 = {}
            tw2 = {}
            for ki in range(3):
                kc = MC[ki + 1] - MC[ki]
                for mi in range(3):
                    mc = MC[mi + 1] - MC[mi]
                    t = wsb.tile([kc, mc], F32R, tag=f"q2_{ki}_{mi}")
                    nc.gpsimd.dma_start(t[:, :], qzw2[MC[ki]:MC[ki + 1], MC[mi]:MC[mi + 1]])
                    tq2[ki, mi] = t
                    t = wsb.tile([kc, mc], FP16, tag=f"w2_{ki}_{mi}")
                    nc.gpsimd.dma_start(t[:, :], w2[MC[ki]:MC[ki + 1], MC[mi]:MC[mi + 1]])
                    tw2[ki, mi] = t
            tzmw, tzlw, tcw = {}, {}, {}
            for ki in range(3):
                kc = MC[ki + 1] - MC[ki]
                t = wsb.tile([kc, L], F32R, tag=f"zmw{ki}")
                nc.gpsimd.dma_start(t[:, :], zmw[MC[ki]:MC[ki + 1], :])
                tzmw[ki] = t
                t = wsb.tile([kc, L], F32R, tag=f"zlw{ki}")
                nc.gpsimd.dma_start(t[:, :], zlw[MC[ki]:MC[ki + 1], :])
                tzlw[ki] = t
                t = wsb.tile([kc, DCORE * 32], FP16, tag=f"cw{ki}")
                nc.gpsimd.dma_start(t[:, :], cwf[MC[ki]:MC[ki + 1], :])
                tcw[ki] = t
            tb1, tb2q, tb2 = {}, {}, {}
            for mi in range(3):
                mc = MC[mi + 1] - MC[mi]
                t = wsb.tile([mc, 1], FP32, tag=f"b1_{mi}")
                nc.gpsimd.dma_start(t[:, :], qzb1[MC[mi]:MC[mi + 1], :])
                tb1[mi] = t
                t = wsb.tile([mc, 1], FP32, tag=f"b2q_{mi}")
                nc.gpsimd.dma_start(t[:, :], qzb2[MC[mi]:MC[mi + 1], :])
                tb2q[mi] = t
                t = wsb.tile([mc, 1], FP32, tag=f"b2_{mi}")
                nc.gpsimd.dma_start(t[:, :], b2[MC[mi]:MC[mi + 1], :])
                tb2[mi] = t
            tzmb = wsb.tile([L, 1], FP32, tag="zmb")
            nc.gpsimd.dma_start(tzmb[:, :], zmb[:, :])
            tzlb = wsb.tile([L, 1], FP32, tag="zlb")
            nc.gpsimd.dma_start(tzlb[:, :], zlb[:, :])
            thzlb = wsb.tile([L, 1], FP32, tag="hzlb")
            nc.gpsimd.dma_start(thzlb[:, :], hzlb[:, :])
            teps = wsb.tile([L, B], FP32, tag="eps")
            nc.gpsimd.dma_start(teps[:, :], epst[:, :])
            tw1 = wsb.tile([128, H], FP16, tag="w1r")
            nc.gpsimd.dma_start(tw1[:, :], w1r[:, :])
            tw4 = wsb.tile([128, NGROUPS], FP32, tag="w4")
            nc.gpsimd.dma_start(tw4[:, :], w4[:, :])
            tcb = wsb.tile([128, NGROUPS // 4], FP32, tag="cbf")
            nc.gpsimd.dma_start(tcb[:, :], cbf[:, :])

            # ---- encoder ----
            with tc.tile_pool(name="eps_ps", bufs=1, space="PSUM") as eps_ps:
                h1 = {}
                for mi in range(3):
                    mc = MC[mi + 1] - MC[mi]
                    pt = eps_ps.tile([128, B], FP32, tag=f"ph1_{mi}")
                    for ki in range(4):
                        nc.tensor.matmul(pt[0:mc, :], tq1[ki, mi][:, :], txt[ki][:, :],
                                         start=(ki == 0), stop=(ki == 3))
                    t = hsb.tile([128, B], F32R, tag=f"h1_{mi}")
                    nc.scalar.activation(t[0:mc, :], pt[0:mc, :], AF.Relu,
                                         bias=tb1[mi][:, :])
                    h1[mi] = t
                h2 = {}
                for mi in range(3):
                    mc = MC[mi + 1] - MC[mi]
                    pt = eps_ps.tile([128, B], FP32, tag=f"ph2_{mi}")
                    for ki in range(3):
                        kc = MC[ki + 1] - MC[ki]
                        nc.tensor.matmul(pt[0:mc, :], tq2[ki, mi][:, :], h1[ki][0:kc, :],
                                         start=(ki == 0), stop=(ki == 2))
                    t = hsb.tile([128, B], F32R, tag=f"h2_{mi}")
                    nc.scalar.activation(t[0:mc, :], pt[0:mc, :], AF.Relu,
                                         bias=tb2q[mi][:, :])
                    h2[mi] = t
                # z heads
                pzm = eps_ps.tile([L, B], FP32, tag="pzm")
                pzl = eps_ps.tile([L, B], FP32, tag="pzl")
                for ki in range(3):
                    kc = MC[ki + 1] - MC[ki]
                    nc.tensor.matmul(pzm[:, :], tzmw[ki][:, :], h2[ki][0:kc, :],
                                     start=(ki == 0), stop=(ki == 2))
                for ki in range(3):
                    kc = MC[ki + 1] - MC[ki]
                    nc.tensor.matmul(pzl[:, :], tzlw[ki][:, :], h2[ki][0:kc, :],
                                     start=(ki == 0), stop=(ki == 2))
                tzm = hsb.tile([L, B], FP32, tag="tzm")
                nc.scalar.activation(tzm[:, :], pzm[:, :], AF.Identity, bias=tzmb[:, :])
                tzlv = hsb.tile([L, B], FP32, tag="tzlv")
                nc.scalar.activation(tzlv[:, :], pzl[:, :], AF.Identity, bias=tzlb[:, :])
                texp = hsb.tile([L, B], FP32, tag="texp")
                nc.scalar.activation(texp[:, :], pzl[:, :], AF.Exp, bias=thzlb[:, :],
                                     scale=0.5)
                # z = z_mean + eps * exp(0.5*zlv)
                tz1 = hsb.tile([L, B], FP32, tag="tz1")
                nc.vector.tensor_tensor(tz1[:, :], teps[:, :], texp[:, :],
                                        op=ALU.mult)
                nc.vector.tensor_tensor(tz1[:, :], tz1[:, :], tzm[:, :],
                                        op=ALU.add)
                # outputs
                nc.sync.dma_start(zt_o[:, :], tz1[:, :])
                nc.sync.dma_start(zmt_o[:, :], tzm[:, :])
                nc.sync.dma_start(zlvt_o[:, :], tzlv[:, :])
                # z4: 4 stacked copies of zT
                tz4 = hsb.tile([128, B], FP32, tag="tz4")
                for i in range(4):
                    nc.sync.dma_start(tz4[32 * i:32 * i + 32, :], tz1[:, :])

            # ---- decoder ----
            def relu_split(dst, src, eng, bias=None):
                if eng == 0:
                    if bias is None:
                        nc.scalar.activation(dst, src, AF.Relu)
                    else:
                        nc.scalar.activation(dst, src, AF.Relu, bias=bias)
                else:
                    if bias is None:
                        nc.vector.tensor_scalar_max(dst, src, 0.0)
                    else:
                        nc.vector.tensor_scalar(dst, src, bias, 0.0,
                                                op0=ALU.add, op1=ALU.max)

            with (
                tc.tile_pool(name="dps", bufs=1, space="PSUM") as dps,
                tc.tile_pool(name="fps", bufs=1, space="PSUM") as fps,
            ):
                wslot = [0]

                def wtile(name):
                    t = dps.tile([128, B], FP32, tag=f"w{wslot[0] % 6}", name=name)
                    wslot[0] += 1
                    return t
                def make_msk(g):
                    m = dec.tile([128, B], FP16, tag="msk", name=f"msk_{g}")
                    nc.vector.tensor_scalar_mul(m[:, :], tz4[:, :], tw4[:, g:g + 1])
                    return m

                msk = make_msk(0)
                for G in range(NGROUPS // 4):
                    finb2 = [fps.tile([128, B], FP32, tag=f"fin{i}", name=f"fin_{G}_{i}")
                             for i in range(2)]
                    finb = [finb2[0], finb2[0], finb2[1], finb2[1]]
                    for gs in range(4):
                        g = 4 * G + gs

                        # L1: 4-row-packed fp16 (K=32 each); g1 free-blocked by mi
                        g1 = [dec.tile([128, 3 * B], FP16, tag=f"g1_{i}", name=f"g1t_{g}_{i}")
                              for i in range(4)]
                        for mi in range(3):
                            mc = MC[mi + 1] - MC[mi]
                            pts = []
                            for i in range(4):
                                pt = wtile(f"l1p_{g}_{mi}_{i}")
                                kw = {}
                                if i == 3:
                                    kw["tile_position"] = (96, 0)
                                nc.tensor.matmul(pt[0:mc, :],
                                                 tw1[32 * i:32 * i + 32, MC[mi]:MC[mi + 1]],
                                                 msk[32 * i:32 * i + 32, :],
                                                 start=True, stop=True, **kw)
                                pts.append(pt)
                            for i in range(4):
                                relu_split(g1[i][0:mc, B * mi:B * mi + B], pts[i][0:mc, :],
                                           (i + mi) % 2)

                        if g + 1 < NGROUPS:
                            nxt_msk = make_msk(g + 1)

                        # L2: fp16, weights shared across the 4 columns (i innermost)
                        g2 = [dec.tile([128, 3 * B], FP16, tag=f"g2_{i}", name=f"g2t_{g}_{i}")
                              for i in range(4)]
                        for mi in range(3):
                            mc = MC[mi + 1] - MC[mi]
                            pts = [wtile(f"l2p_{g}_{mi}_{i}") for i in range(4)]
                            for ki in range(3):
                                kc = MC[ki + 1] - MC[ki]
                                for i in range(4):
                                    nc.tensor.matmul(pts[i][0:mc, :], tw2[ki, mi][:, :],
                                                     g1[i][0:kc, B * ki:B * ki + B],
                                                     start=(ki == 0), stop=(ki == 2))
                            for i in range(4):
                                relu_split(g2[i][0:mc, B * mi:B * mi + B], pts[i][0:mc, :],
                                           (i + mi) % 2, bias=tb2[mi][:, :])

                        # final: fp16 matvecs, strip i -> own bank, M=32 rows with
                        # zero-padded lhsT; 4 groups accumulate before copy-out
                        for ki in range(3):
                            kc = MC[ki + 1] - MC[ki]
                            for i in (0, 2, 1, 3):
                                jloc = 4 * g + i
                                kw = {}
                                if i == 3:
                                    kw["tile_position"] = (0, 96)
                                elif i > 0:
                                    kw["tile_position"] = (0, 32 * i)
                                nc.tensor.matmul(finb[i][32 * i:32 * i + 32, :],
                                                 tcw[ki][:, 32 * jloc:32 * jloc + 32],
                                                 g2[i][0:kc, B * ki:B * ki + B],
                                                 start=(gs == 0 and ki == 0),
                                                 stop=(gs == 3 and ki == 2), **kw)

                        if g + 1 < NGROUPS:
                            msk = nxt_msk

                    # copy out the 4 accumulated banks (+ col_b) and store
                    xr = dec.tile([128, B], FP32, tag="xr", name=f"xr_{G}")
                    for i in range(4):
                        nc.vector.tensor_scalar_add(
                            xr[32 * i:32 * i + 4, :], finb[i][32 * i:32 * i + 4, :],
                            tcb[32 * i:32 * i + 4, G:G + 1])
                        nc.sync.dma_start(xpart[16 * G + i:16 * G + 16:4, :],
                                          xr[32 * i:32 * i + 4, :])

    _split_sync_waits(nc)
    return nc


_cached_nc = None


def kernel(x, eps, W, qz_w1, qz_b1, qz_w2, qz_b2, zm_w, zm_b, zl_w, zl_b,
           gen_w1, gen_w2, gen_b2, col_w, col_b):
    global _cached_nc
    f32 = np.float32

    x = np.asarray(x, dtype=f32)
    eps = np.asarray(eps, dtype=f32)
    W = np.asarray(W, dtype=f32)
    qz_w1 = np.asarray(qz_w1, dtype=f32)
    qz_b1 = np.asarray(qz_b1, dtype=f32)
    qz_w2 = np.asarray(qz_w2, dtype=f32)
    qz_b2 = np.asarray(qz_b2, dtype=f32)
    zm_w = np.asarray(zm_w, dtype=f32)
    zm_b = np.asarray(zm_b, dtype=f32)
    zl_w = np.asarray(zl_w, dtype=f32)
    zl_b = np.asarray(zl_b, dtype=f32)
    gen_w1 = np.asarray(gen_w1, dtype=f32)
    gen_w2 = np.asarray(gen_w2, dtype=f32)
    gen_b2 = np.asarray(gen_b2, dtype=f32)
    col_w = np.asarray(col_w, dtype=f32)
    col_b = np.asarray(col_b, dtype=f32)

    shared = {
        "xt": np.ascontiguousarray(x.T).astype(np.float16),
        "epst": np.ascontiguousarray(eps.T),
        "qzw1": qz_w1.astype(np.float16),
        "qzb1": qz_b1.reshape(H, 1),
        "qzw2": qz_w2,
        "qzb2": qz_b2.reshape(H, 1),
        "zmw": zm_w,
        "zmb": zm_b.reshape(L, 1),
        "zlw": zl_w,
        "zlb": zl_b.reshape(L, 1),
        "hzlb": (0.5 * zl_b).reshape(L, 1),
        "w1r": np.tile(gen_w1, (4, 1)).astype(np.float16),
        "w2": gen_w2.astype(np.float16),
        "b2": gen_b2.reshape(H, 1),
    }

    in_maps = []
    for c in range(NCORES):
        sl = slice(c * DCORE, (c + 1) * DCORE)
        Wc = W[sl]                                 # [64, 32]
        w4c = np.ascontiguousarray(
            Wc.reshape(NGROUPS, 4, L).transpose(1, 2, 0).reshape(128, NGROUPS))
        cwc = np.zeros((H, DCORE * 32), dtype=np.float16)
        for j in range(DCORE):
            r = (j // 4) % 4
            cwc[:, 32 * j + r] = col_w[c * DCORE + j].astype(np.float16)
        cbc = np.zeros((128, NGROUPS // 4), dtype=f32)
        for Gs in range(NGROUPS // 4):
            for r in range(4):
                for i in range(4):
                    cbc[32 * i + r, Gs] = col_b[c * DCORE + 16 * Gs + 4 * r + i]
        m = dict(shared)
        m.update({"w4": w4c, "cwf": cwc, "cbf": cbc})
        in_maps.append(m)

    if _cached_nc is None:
        _cached_nc = _build_program()

    res = run_bass_kernel_spmd(_cached_nc, in_maps, core_ids=list(range(NCORES)))
    kernel._last_results = res

    x_mean = np.concatenate([res.results[c]["xpart"] for c in range(NCORES)], axis=0).T
    z = res.results[0]["zt_o"].T
    z_mean = res.results[0]["zmt_o"].T
    z_log_var = res.results[0]["zlvt_o"].T
    return (np.ascontiguousarray(x_mean), np.ascontiguousarray(z),
            np.ascontiguousarray(z_mean), np.ascontiguousarray(z_log_var))
